# revision 19
# baseline (speedup 1.0000x reference)
"""Trainium2 Bass kernel for nn_ConstraintsModule (v3.1).

Reference math:
    m = preds[:, atoms]                                   # [B, N]
    body_rev[b,c,j] = pos_body[c,j] + m[b,j]*(neg_body-pos_body)[c,j]
    body_min[b,c]   = 1 - max_j body_rev[b,c,j]
    lb[b,n] = max_c body_min[b,c]*pos_head[c,n]
    ub[b,n] = 1 - max_c body_min[b,c]*neg_head[c,n]
    updated = clamp(m, min(lb,ub), max(lb,ub))
    out = preds with columns `atoms` replaced by updated

Min-form rewrite: body_min = min over body literals of (m_j if pos literal,
1-m_j if neg literal), padded with 1.0.  Packed literals carry RELATIVE
error <= 2^-9 in bf16 and min/max preserve relative error, so the result is
within ~4e-3 relative of the fp32 reference (gate 2e-2).

Measured DVE rates (TRN2): tensor_reduce = 1.04 ns/elem regardless of dtype
or stride; tensor_tensor bf16 = 0.53 ns/elem.  Stage 1 therefore packs each
slot's literals into two aligned half-arrays A|B, takes one contiguous
bf16 tensor_tensor(min) per chunk, and reduces only the halved array.

Device pipeline per core (batch rows on partitions):
  1. DMA G [128, COLS] bf16 in 4 chunks over 3 queues (scalar, gpsimd,
     sync), smallest chunk first; each chunk = [A-block | B-block] with
     per-slot half-widths bucketed (pad 1.0, dummy slots 0.0).
  2. Stage 1 (DVE): per chunk tensor_tensor(min) A,B -> t, then per
     width-bucket strided tensor_reduce(min) -> bmin [128, S_pad] bf16.
  3. PE transpose of bmin (two K-tiles split at a chunk boundary so the
     first transpose+matmul overlaps the remaining reduces) + matmul with
     one-hot P[slot, cell] -> PSUM [128, 528] fp32, member-major cells
     (cell = member*132 + sign*66 + local atom).  Head bins capped at 4
     members; oversized bins spill members 5..8 into donor columns 64/65,
     folded back by one max.
  4. DVE max sweeps (contiguous) -> lb/ubm, then the fp32 clamp chain with
     exact m values; DMA out updated [128, 66].
ACT pre-warms its table and stages PSUM->SBUF copies; PE is pre-warmed.

Sharding: 64 atoms per core dealt greedily by literal load; one SPMD
program for all cores (bucket counts padded to the cross-core max).
"""

import sys
from contextlib import ExitStack

import numpy as np

if "/opt/trn_rl_repo" not in sys.path:
    sys.path.insert(0, "/opt/trn_rl_repo")

import concourse.bacc as bacc
import concourse.tile as tile
from concourse import mybir
from concourse.bass_utils import run_bass_kernel_spmd
from concourse.masks import make_identity

B = 128
C = 1024
N = 512
NCORES = 8
NA = N // NCORES + 2      # 64 real atoms + 2 donor columns = 66
W_BIN = 4
CELLS = W_BIN * 2 * NA    # 528, cell = (sign*66 + a_local)*4 + member
NBINS = 2 * NA            # 132

_TRACE = False
_LAST_RESULTS = None
_PROGRAM_CACHE: dict = {}

_BF_NP = mybir.dt.np(mybir.dt.bfloat16)
_F8_NP = mybir.dt.np(mybir.dt.float8e4)


def _build_program(key):
    """key = (chunks, s_pad, t_cols, split).  chunks = tuple of
    (c0, c1, toff, reduces); reduces = tuple of (toff, nslots, w, boff).
    Each chunk's G range is [A | B] halves of equal width (c1-c0)/2."""
    if key in _PROGRAM_CACHE:
        return _PROGRAM_CACHE[key]
    chunks, s_pad, t_cols, splits = key
    ksl = [0] + list(splits) + [s_pad]
    assert all(0 < ksl[i + 1] - ksl[i] <= 128 for i in range(len(ksl) - 1))
    cols = chunks[-1][1]

    dt = mybir.dt
    nc = bacc.Bacc(
        "TRN2", target_bir_lowering=False, debug=False, enable_partition_id=False
    )
    c_ds = [
        nc.dram_tensor(f"c{i}", [B, c1 - c0], dt.bfloat16, kind="ExternalInput")
        for i, (c0, c1, _, _) in enumerate(chunks)
    ]
    p_ds = [
        nc.dram_tensor(
            f"p{j}", [ksl[j + 1] - ksl[j], CELLS], dt.float8e4,
            kind="ExternalInput",
        )
        for j in range(len(ksl) - 1)
    ]
    mloc_d = nc.dram_tensor("mloc", [B, NA], dt.float32, kind="ExternalInput")
    out_d = nc.dram_tensor("upd", [B, NA], dt.float32, kind="ExternalOutput")

    with ExitStack() as ctx:
        tc = ctx.enter_context(tile.TileContext(nc))
        pool = ctx.enter_context(tc.tile_pool(name="main", bufs=1))
        psum = ctx.enter_context(tc.tile_pool(name="ps", bufs=1, space="PSUM"))

        g_sb = pool.tile([B, cols], dt.bfloat16, tag="g")
        t_sb = pool.tile([B, t_cols], dt.bfloat16, tag="t")
        p_sbs = [
            pool.tile([ksl[j + 1] - ksl[j], CELLS], dt.float8e4, name=f"p{j}sb", tag=f"p{j}")
            for j in range(len(ksl) - 1)
        ]
        mloc_sb = pool.tile([B, NA], dt.float32, tag="mloc")
        bmin_sb = pool.tile([B, s_pad], dt.bfloat16, tag="bmin")
        ident = pool.tile([128, 128], dt.bfloat16, tag="ident")
        warm = pool.tile([B, 8], dt.float32, tag="warm")

        # G chunks spread over three queues (scalar, gpsimd, sync) so each
        # is in flight early; P and mloc follow chunk 2 on the sync queue.
        dma_engines = [nc.scalar, nc.sync, nc.gpsimd, nc.scalar]
        for i, (c0, c1, _, _) in enumerate(chunks):
            dma_engines[i % 4].dma_start(g_sb[:, c0:c1], c_ds[i].ap())
        for j, p_sb in enumerate(p_sbs):
            nc.sync.dma_start(p_sb[:], p_ds[j].ap())
        nc.sync.dma_start(mloc_sb[:], mloc_d.ap())

        # Warm-ups while DMAs fly: ACT table load, PE p-state, identity.
        make_identity(nc, ident[:])
        nc.scalar.copy(warm[:], warm[:])
        wpt = psum.tile([8, 128], dt.bfloat16, tag="wpt")
        nc.tensor.transpose(wpt[:], ident[:, 0:8], ident[:])

        for i, (c0, c1, toff, reduces) in enumerate(chunks):
            half = (c1 - c0) // 2
            nc.vector.tensor_tensor(
                t_sb[:, toff : toff + half],
                g_sb[:, c0 : c0 + half],
                g_sb[:, c0 + half : c1],
                op=mybir.AluOpType.min,
            )
            for toff_r, nslots, w, boff in reduces:
                t3 = t_sb[:, toff_r : toff_r + nslots * w].rearrange(
                    "p (c k) -> p c k", k=w
                )
                nc.vector.tensor_reduce(
                    bmin_sb[:, boff : boff + nslots], t3,
                    axis=mybir.AxisListType.X, op=mybir.AluOpType.min,
                )
            # keep the PE clocked up while reduces run
            nc.tensor.transpose(wpt[:], bmin_sb[:, 0:8], ident[:])

        # PE scatter: bminT (K-tiles at chunk boundaries, so K0's
        # transpose+matmul overlap the remaining reduces) @ one-hot P ->
        # bin-major cells in PSUM.
        po = psum.tile([B, CELLS], dt.float32, tag="po")
        nk = len(ksl) - 1
        for j in range(nk):
            k0, k1 = ksl[j], ksl[j + 1]
            ptj = psum.tile([k1 - k0, 128], dt.bfloat16, name=f"pt{j}", tag=f"pt{j}")
            nc.tensor.transpose(ptj[:], bmin_sb[:, k0:k1], ident[:])
            btj = pool.tile([k1 - k0, 128], dt.bfloat16, name=f"bt{j}", tag=f"bt{j}")
            nc.scalar.copy(btj[:], ptj[:])
            for nt0 in range(0, CELLS, 512):
                nt1 = min(nt0 + 512, CELLS)
                nc.tensor.matmul(
                    po[:, nt0:nt1], btj[:], p_sbs[j][:, nt0:nt1],
                    start=(j == 0), stop=(j == nk - 1),
                )

        # One strided max-reduce over the member dim: [B, 132, 4] -> [B, 132]
        po3 = po[:].rearrange("p (c k) -> p c k", k=W_BIN)
        lbub = pool.tile([B, NBINS], dt.float32, tag="lbub")
        nc.vector.tensor_reduce(
            lbub[:], po3, axis=mybir.AxisListType.X, op=mybir.AluOpType.max
        )
        # Fold donor columns (oversized-bin spill) back into atoms 0/1.
        nc.vector.tensor_tensor(
            lbub[:, 0:2], lbub[:, 0:2], lbub[:, NA - 2 : NA],
            op=mybir.AluOpType.max,
        )

        # updated = max(min(lb, ub), min(max(lb, ub), m)),  ub = 1 - ubm
        lb = lbub[:, 0:NA]
        ubm = lbub[:, NA:NBINS]
        ub_sb = pool.tile([B, NA], dt.float32, tag="ub")
        nc.vector.tensor_scalar(
            ub_sb[:], ubm, -1.0, 1.0,
            op0=mybir.AluOpType.mult, op1=mybir.AluOpType.add,
        )
        lo_sb = pool.tile([B, NA], dt.float32, tag="lo")
        nc.vector.tensor_tensor(lo_sb[:], lb, ub_sb[:], op=mybir.AluOpType.min)
        hi_sb = pool.tile([B, NA], dt.float32, tag="hi")
        nc.vector.tensor_tensor(hi_sb[:], lb, ub_sb[:], op=mybir.AluOpType.max)
        upd_sb = pool.tile([B, NA], dt.float32, tag="upd")
        nc.vector.tensor_tensor(
            upd_sb[:], hi_sb[:], mloc_sb[:], op=mybir.AluOpType.min
        )
        nc.vector.tensor_tensor(
            upd_sb[:], lo_sb[:], upd_sb[:], op=mybir.AluOpType.max
        )
        nc.sync.dma_start(out_d.ap(), upd_sb[:])

    nc.compile()
    _PROGRAM_CACHE[key] = nc
    return nc


def _plan_buckets(kcounts):
    """DP over half-width cut points minimizing DVE cost: padded half
    columns * (1.04 reduce + 0.53 tt) + ~150ns per reduce instruction."""
    ws = sorted(kcounts)
    nw = len(ws)
    suffix_cnt = [0] * (nw + 1)
    for i in range(nw - 1, -1, -1):
        suffix_cnt[i] = suffix_cnt[i + 1] + kcounts[ws[i]]
    best = {}

    def solve(i):
        if i >= nw:
            return (0.0, ())
        if i in best:
            return best[i]
        r = None
        for j in range(i, nw):
            cnt = suffix_cnt[i] - suffix_cnt[j + 1]
            cost = cnt * ws[j] * 1.57 / NCORES + 150.0
            sub = solve(j + 1)
            tot = cost + sub[0]
            if r is None or tot < r[0]:
                r = (tot, (ws[j],) + sub[1])
        best[i] = r
        return r

    return solve(0)[1]


def kernel(preds, pos_head, neg_head, pos_body, neg_body, atoms):
    global _LAST_RESULTS
    preds = np.ascontiguousarray(np.asarray(preds, dtype=np.float32))
    pos_head = np.asarray(pos_head)
    neg_head = np.asarray(neg_head)
    pos_body = np.asarray(pos_body)
    neg_body = np.asarray(neg_body)
    atoms_np = np.asarray(atoms).astype(np.int64)

    m = np.ascontiguousarray(preds[:, atoms_np].astype(np.float32))  # [B, N]
    rev = (np.float32(1.0) - m).astype(np.float32)
    # literal cols: [0,N) = m, [N,2N) = 1-m, 2N = 0.0 (dummy), 2N+1 = 1.0 (pad)
    mext = np.concatenate(
        [m, rev, np.zeros((B, 1), np.float32), np.ones((B, 1), np.float32)],
        axis=1,
    ).astype(_BF_NP)
    DUMMY_COL, PAD_COL = 2 * N, 2 * N + 1

    pb = pos_body != 0
    nb_ = neg_body != 0
    lit_cols = []
    kh = np.zeros(C, np.int64)  # half width, rounded to 2
    for c in range(C):
        jp = np.nonzero(pb[c])[0]
        jn = np.nonzero(nb_[c])[0]
        lit_cols.append(np.concatenate([jp, N + jn]))
        kh[c] = max(-(-(-(-(jp.size + jn.size) // 2)) // 2) * 2, 2)

    from collections import Counter, defaultdict

    cuts = _plan_buckets(Counter(int(w) for w in kh))
    wh = np.zeros(C, np.int64)  # bucketed half width
    for c in range(C):
        wh[c] = next(w for w in sorted(cuts) if w >= kh[c])

    ph_atom = pos_head.argmax(1)
    ph_has = pos_head.max(1) > 0
    nh_atom = neg_head.argmax(1)
    nh_has = neg_head.max(1) > 0
    pos_bins = [[] for _ in range(N)]
    neg_bins = [[] for _ in range(N)]
    for c in np.nonzero(ph_has)[0]:
        pos_bins[ph_atom[c]].append(c)
    for c in np.nonzero(nh_has)[0]:
        neg_bins[nh_atom[c]].append(c)
    assert max(len(b) for b in neg_bins) <= W_BIN
    big_atoms = [a for a in range(N) if len(pos_bins[a]) > W_BIN]
    assert all(len(pos_bins[a]) <= 2 * W_BIN for a in big_atoms)

    weight = np.array(
        [sum(wh[c] for c in pos_bins[a] + neg_bins[a]) for a in range(N)]
    )
    foot = [Counter(int(wh[c]) for c in pos_bins[a] + neg_bins[a]) for a in range(N)]
    order = sorted(range(N), key=lambda a: (a not in big_atoms, -weight[a]))
    core_load = np.zeros(NCORES, np.int64)
    core_atoms = [[] for _ in range(NCORES)]
    nbig = np.zeros(NCORES, np.int64)
    cntw = [Counter() for _ in range(NCORES)]  # per-core bucket counts
    maxw = Counter()                           # cross-core max per bucket
    for a in order:
        big = a in big_atoms
        cands = [
            k for k in range(NCORES)
            if len(core_atoms[k]) < 64 and (not big or nbig[k] < 2)
        ]
        # padded-column increase if atom a goes to core k
        def pad_inc(k):
            return sum(
                w * max(0, cntw[k][w] + f - maxw[w]) for w, f in foot[a].items()
            )
        k = min(cands, key=lambda k: (pad_inc(k), core_load[k]))
        core_atoms[k].append(int(a))
        core_load[k] += weight[a]
        for w, f in foot[a].items():
            cntw[k][w] += f
            maxw[w] = max(maxw[w], cntw[k][w])
        if big:
            nbig[k] += 1

    # Per-core slots by half-width bucket.  cell = member*NBINS + sign*NA +
    # a_local; pos members 4..7 of big atoms go to donor column 64/65.
    core_buckets = []
    for k in range(NCORES):
        bk = defaultdict(list)
        for a_local, a in enumerate(core_atoms[k]):
            for mem, c in enumerate(pos_bins[a]):
                if mem < W_BIN:
                    cell = a_local * W_BIN + mem
                else:
                    assert a_local < 2
                    cell = (NA - 2 + a_local) * W_BIN + (mem - W_BIN)
                bk[int(wh[c])].append((lit_cols[c], cell))
            for mem, c in enumerate(neg_bins[a]):
                bk[int(wh[c])].append(
                    (lit_cols[c], (NA + a_local) * W_BIN + mem)
                )
        core_buckets.append(bk)
    all_w = sorted({w for bk in core_buckets for w in bk})
    bucket_cnt = {
        w: max(len(bk.get(w, ())) for bk in core_buckets) for w in all_w
    }
    s_pad = sum(bucket_cnt.values())
    t_cols = sum(bucket_cnt[w] * w for w in all_w)

    # Chunks at slot boundaries (t-column space); G columns are 2x.
    slot_edges = []
    toff = 0
    for w in all_w:
        for _ in range(bucket_cnt[w]):
            toff += w
            slot_edges.append(toff)
    fracs = (0.08, 0.45)
    cutpts = sorted(
        {min(slot_edges, key=lambda e: abs(e - int(t_cols * f))) for f in fracs}
    )
    cutpts = [0] + [cp for cp in cutpts if 0 < cp < t_cols] + [t_cols]

    chunks = []  # (g0, g1, toff, reduces)
    toff = 0
    boff = 0
    ci = 0
    cur_reduces = []
    cur_t0 = 0
    chunk_ends = []
    for w in all_w:
        nsl = bucket_cnt[w]
        s = 0
        while s < nsl:
            take = min((cutpts[ci + 1] - toff) // w, nsl - s)
            if take > 0:
                cur_reduces.append((toff, take, w, boff))
                toff += take * w
                boff += take
                s += take
            if toff >= cutpts[ci + 1] and ci + 2 <= len(cutpts) - 1:
                chunks.append((2 * cur_t0, 2 * toff, cur_t0, tuple(cur_reduces)))
                chunk_ends.append(boff)
                cur_t0 = toff
                cur_reduces = []
                ci += 1
    chunks.append((2 * cur_t0, 2 * t_cols, cur_t0, tuple(cur_reduces)))
    chunks = tuple(c for c in chunks if c[1] > c[0])
    # K-tile split after the second chunk: K0's transpose+matmul overlap
    # the remaining reduces, K1 runs once at the end.
    splits = [b for b in chunk_ends[1:2] if 0 < b < s_pad]
    ksl = [0] + splits + [s_pad]
    if any(ksl[i + 1] - ksl[i] > 128 for i in range(len(ksl) - 1)):
        splits = sorted(set(b for b in chunk_ends if 0 < b < s_pad))
        ksl = [0] + splits + [s_pad]
        assert all(ksl[i + 1] - ksl[i] <= 128 for i in range(len(ksl) - 1))
    splits = tuple(splits)

    key = (chunks, s_pad, t_cols, splits)
    nc = _build_program(key)

    in_maps = []
    for k in range(NCORES):
        col_idx = np.full(2 * t_cols, DUMMY_COL, np.int32)
        P = np.zeros((s_pad, CELLS), _F8_NP)
        si = 0
        slot_w = []  # (w, slot or None) in slot order
        for w in all_w:
            slots = core_buckets[k].get(w, [])
            for j in range(bucket_cnt[w]):
                slot_w.append((w, slots[j] if j < len(slots) else None))
        # fill G: per chunk [A block | B block]
        for g0, g1, toff_c, reduces in chunks:
            half = (g1 - g0) // 2
            a_off = g0
            b_off = g0 + half
            for toff_r, nslots, w, boff_r in reduces:
                for j in range(nslots):
                    wslot = slot_w[boff_r + j]
                    assert wslot[0] == w
                    if wslot[1] is not None:
                        lc, cell = wslot[1]
                        h = -(-lc.size // 2)
                        col_idx[a_off : a_off + h] = lc[0:h]
                        col_idx[a_off + h : a_off + w] = PAD_COL
                        col_idx[b_off : b_off + (lc.size - h)] = lc[h:]
                        col_idx[b_off + (lc.size - h) : b_off + w] = PAD_COL
                        P[boff_r + j, cell] = 1.0
                    a_off += w
                    b_off += w
        g = np.ascontiguousarray(mext[:, col_idx])
        ml = np.zeros((B, NA), np.float32)
        ml[:, 0:64] = m[:, core_atoms[k]]
        ksl_h = [0] + list(splits) + [s_pad]
        im = {"mloc": ml}
        for j in range(len(ksl_h) - 1):
            im[f"p{j}"] = np.ascontiguousarray(P[ksl_h[j] : ksl_h[j + 1]])
        for i, (g0, g1, _, _) in enumerate(chunks):
            im[f"c{i}"] = np.ascontiguousarray(g[:, g0:g1])
        in_maps.append(im)

    res = run_bass_kernel_spmd(
        nc, in_maps, core_ids=list(range(NCORES)), trace=_TRACE
    )
    _LAST_RESULTS = res

    out = preds.copy()
    for k in range(NCORES):
        out[:, atoms_np[core_atoms[k]]] = res.results[k]["upd"][:, 0:64]
    return out
